# revision 4
# baseline (speedup 1.0000x reference)
"""Trainium2 Bass kernel for nn_Aggregation (SAN-style local aggregation).

out[n, g*32+cc, h, w] = sum_{kh,kw} input[n, g*32+cc, h-3+kh, w-3+kw] * weight[n, cc, kh*7+kw, h, w]

Sharding: data-parallel over batch N=16 across 8 NeuronCores (2 images/core).

Per-core layout:
  partition p = cc*4 + blk   (cc in [0,32): weight channel, blk in [0,4): block of 8 output rows)
  in_pad[p][n, g, r, col] = zero-padded input rows [blk*8-3, blk*8+11), cols [-3, 35)
  w_t[p][n, kk, hb, w]    = weight[n, cc, kk, blk*8+hb, w]
  For each tap kk=(kh,kw): acc[p][n,g,hb,w] += in_pad[p][n,g,hb+kh,w+kw] * w_t[p][n,kk,hb,w]
  (weight broadcast over g via stride-0 access pattern)
"""

import numpy as np

N, C, H, W = 16, 256, 32, 32
K, PAD = 7, 3
CC, G = 32, 8
KK = K * K
NCORES = 8
NPC = N // NCORES
BLK, HB = 4, 8
R, COLP = HB + 2 * PAD, W + 2 * PAD  # 14, 38

_cache = {}


def _build():
    import concourse.bacc as bacc
    import concourse.mybir as mybir
    import concourse.tile as tile

    fp32 = mybir.dt.float32
    mult = mybir.AluOpType.mult
    add = mybir.AluOpType.add

    nc = bacc.Bacc("TRN2", target_bir_lowering=False, debug=False, num_devices=NCORES)
    x = nc.dram_tensor("input", [NPC, C, H, W], fp32, kind="ExternalInput").ap()
    wgt = nc.dram_tensor("weight", [NPC, CC, KK, H, W], fp32, kind="ExternalInput").ap()
    y = nc.dram_tensor("output", [NPC, C, H, W], fp32, kind="ExternalOutput").ap()

    with tile.TileContext(nc) as tc:
        with (
            tc.tile_pool(name="main", bufs=1) as pool,
            tc.tile_pool(name="prod", bufs=2) as ppool,
        ):
            in_pad = pool.tile([128, NPC, G, R, COLP], fp32)
            stage = pool.tile([128, NPC, G, R, W], fp32)
            w_t = pool.tile([128, NPC, KK, HB, W], fp32)
            acc = pool.tile([128, NPC, G, HB, W], fp32)

            nc.gpsimd.memset(in_pad[:].rearrange("p n g r c -> p (n g r c)"), 0.0)
            nc.gpsimd.memset(stage[:].rearrange("p n g r c -> p (n g r c)"), 0.0)

            for n in range(NPC):
                for blk in range(BLK):
                    # rows of the padded window [blk*8-3, blk*8+11) that exist
                    h0 = max(0, blk * HB - PAD)
                    h1 = min(H, blk * HB + HB + PAD)
                    r0 = h0 - (blk * HB - PAD)
                    dst = stage[blk::BLK, n].rearrange("p g r c -> p g (r c)")[
                        :, :, r0 * W : (r0 + (h1 - h0)) * W
                    ]
                    src = x[n].rearrange("(g cc) h w -> cc g (h w)", g=G)[
                        :, :, h0 * W : h1 * W
                    ]
                    nc.sync.dma_start(out=dst, in_=src)
                # place unpadded rows into the column-padded tile (ACT is idle)
                nc.scalar.copy(
                    out=in_pad[:, n, :, :, PAD : PAD + W], in_=stage[:, n]
                )
                wsrc = wgt[n].rearrange("cc kk (blk hb) w -> cc blk kk (hb w)", blk=BLK)
                for blk in range(BLK):
                    nc.sync.dma_start(
                        out=w_t[blk::BLK, n].rearrange("p kk hb w -> p kk (hb w)"),
                        in_=wsrc[:, blk],
                    )

            for n in range(NPC):
                for kh in range(K):
                    for kw in range(K):
                        kk = kh * K + kw
                        in0 = in_pad[:, n, :, kh : kh + HB, kw : kw + W]
                        in1 = (
                            w_t[:, n, kk]
                            .unsqueeze(1)
                            .broadcast_to([128, G, HB, W])
                        )
                        if kk == 0:
                            nc.vector.tensor_tensor(
                                out=acc[:, n], in0=in0, in1=in1, op=mult
                            )
                        else:
                            prod = ppool.tile([128, G, HB, W], fp32)
                            nc.vector.tensor_tensor(
                                out=prod[:], in0=in0, in1=in1, op=mult
                            )
                            nc.vector.tensor_tensor(
                                out=acc[:, n], in0=acc[:, n], in1=prod[:], op=add
                            )
                dsty = y[n].rearrange(
                    "(g cc) (blk hb) w -> g cc blk (hb w)", g=G, blk=BLK
                )
                for g in range(G):
                    nc.sync.dma_start(
                        out=dsty[g],
                        in_=acc[:, n, g].rearrange("p hb w -> p (hb w)"),
                    )

    nc.compile()
    return nc


def _get_nc():
    if "nc" not in _cache:
        _cache["nc"] = _build()
    return _cache["nc"]


def kernel(input_, weight, _trace=False):
    from concourse.bass_utils import run_bass_kernel_spmd

    nc = _get_nc()
    input_ = np.ascontiguousarray(input_, dtype=np.float32)
    weight = np.ascontiguousarray(weight, dtype=np.float32)
    in_maps = [
        {
            "input": input_[i * NPC : (i + 1) * NPC],
            "weight": weight[i * NPC : (i + 1) * NPC],
        }
        for i in range(NCORES)
    ]
    res = run_bass_kernel_spmd(nc, in_maps, list(range(NCORES)), trace=_trace)
    _cache["last_result"] = res
    out = np.concatenate([res.results[i]["output"] for i in range(NCORES)], axis=0)
    return out


# revision 5
# speedup vs baseline: 1.6340x; 1.6340x over previous
"""Trainium2 Bass kernel for nn_Aggregation (SAN-style local aggregation).

out[n, g*32+cc, h, w] = sum_{kh,kw} input[n, g*32+cc, h-3+kh, w-3+kw] * weight[n, cc, kh*7+kw, h, w]

Sharding: data-parallel over batch N=16 across 8 NeuronCores (2 images/core).

Per-core layout:
  partition p = cc*4 + blk   (cc in [0,32): weight channel, blk in [0,4): block of 8 output rows)
  in_pad[p][n, g, r, col] = zero-padded input rows [blk*8-3, blk*8+11), cols [-3, 35)
  w_t[p][n, kk, hb, w]    = weight[n, cc, kk, blk*8+hb, w]
  For each tap kk=(kh,kw): acc[p][n,g,hb,w] += in_pad[p][n,g,hb+kh,w+kw] * w_t[p][n,kk,hb,w]
  (weight broadcast over g via stride-0 access pattern)

Mode "fp16row" (default): products and within-row (7-tap) accumulation in fp16
on the DVE at 2x rate; row sums flushed into an fp32 accumulator. A second
input copy shifted by one column keeps odd-kw taps 4B-aligned so the DVE's
2x perf mode stays engaged. Max abs error vs fp32 reference ~7e-4 of absmax.
Mode "fp32": everything fp32 (exact, ~2x slower).
"""

import numpy as np

N, C, H, W = 16, 256, 32, 32
K, PAD = 7, 3
CC, G = 32, 8
KK = K * K
NCORES = 8
NPC = N // NCORES
BLK, HB = 4, 8
R, COLP = HB + 2 * PAD, W + 2 * PAD  # 14, 38

MODE = "fp16row"

_cache = {}


def _build(mode):
    import concourse.bacc as bacc
    import concourse.mybir as mybir
    import concourse.tile as tile

    fp32 = mybir.dt.float32
    fp16 = mybir.dt.float16
    cdt = fp32 if mode == "fp32" else fp16  # compute dtype
    mult = mybir.AluOpType.mult
    add = mybir.AluOpType.add

    nc = bacc.Bacc("TRN2", target_bir_lowering=False, debug=False, num_devices=NCORES)
    x = nc.dram_tensor("input", [NPC, C, H, W], fp32, kind="ExternalInput").ap()
    wgt = nc.dram_tensor("weight", [NPC, CC, KK, H, W], fp32, kind="ExternalInput").ap()
    y = nc.dram_tensor("output", [NPC, C, H, W], fp32, kind="ExternalOutput").ap()

    with tile.TileContext(nc) as tc:
        with (
            tc.tile_pool(name="main", bufs=1) as pool,
            tc.tile_pool(name="prod", bufs=2) as ppool,
            tc.tile_pool(name="rowp", bufs=2) as rpool,
        ):
            in_pad = pool.tile([128, NPC, G, R, COLP], cdt)
            stage = pool.tile([128, NPC, G, R, W], cdt)
            w_t = pool.tile([128, NPC, KK, HB, W], cdt)
            acc = pool.tile([128, NPC, G, HB, W], fp32)
            if mode == "fp16row":
                in_pad1 = pool.tile([128, NPC, G, R, COLP], cdt)

            nc.gpsimd.memset(in_pad[:].rearrange("p n g r c -> p (n g r c)"), 0.0)
            nc.gpsimd.memset(stage[:].rearrange("p n g r c -> p (n g r c)"), 0.0)
            if mode == "fp16row":
                nc.gpsimd.memset(
                    in_pad1[:].rearrange("p n g r c -> p (n g r c)"), 0.0
                )

            # DMA engine: SWDGE (gpsimd) is required for dtype-cast loads
            dma_in = nc.gpsimd if cdt != fp32 else nc.sync

            for n in range(NPC):
                for blk in range(BLK):
                    # rows of the padded window [blk*8-3, blk*8+11) that exist
                    h0 = max(0, blk * HB - PAD)
                    h1 = min(H, blk * HB + HB + PAD)
                    r0 = h0 - (blk * HB - PAD)
                    dst = stage[blk::BLK, n].rearrange("p g r c -> p g (r c)")[
                        :, :, r0 * W : (r0 + (h1 - h0)) * W
                    ]
                    src = x[n].rearrange("(g cc) h w -> cc g (h w)", g=G)[
                        :, :, h0 * W : h1 * W
                    ]
                    dma_in.dma_start(out=dst, in_=src)
                # place unpadded rows into the column-padded tile (ACT is idle)
                nc.scalar.copy(out=in_pad[:, n, :, :, PAD : PAD + W], in_=stage[:, n])
                if mode == "fp16row":
                    # column-shifted copy: in_pad1[..., c] = in_pad[..., c+1]
                    # keeps odd-kw taps 4B-aligned for the DVE 2x perf mode
                    nc.scalar.copy(
                        out=in_pad1[:, n, :, :, 0 : COLP - 1],
                        in_=in_pad[:, n, :, :, 1:COLP],
                    )
                wsrc = wgt[n].rearrange("cc kk (blk hb) w -> cc blk kk (hb w)", blk=BLK)
                for blk in range(BLK):
                    dma_in.dma_start(
                        out=w_t[blk::BLK, n].rearrange("p kk hb w -> p kk (hb w)"),
                        in_=wsrc[:, blk],
                    )

            for n in range(NPC):
                if mode == "fp32":
                    for kh in range(K):
                        for kw in range(K):
                            kk = kh * K + kw
                            in0 = in_pad[:, n, :, kh : kh + HB, kw : kw + W]
                            in1 = (
                                w_t[:, n, kk].unsqueeze(1).broadcast_to([128, G, HB, W])
                            )
                            if kk == 0:
                                nc.vector.tensor_tensor(
                                    out=acc[:, n], in0=in0, in1=in1, op=mult
                                )
                            else:
                                prod = ppool.tile([128, G, HB, W], cdt)
                                nc.vector.tensor_tensor(
                                    out=prod[:], in0=in0, in1=in1, op=mult
                                )
                                nc.vector.tensor_tensor(
                                    out=acc[:, n], in0=acc[:, n], in1=prod[:], op=add
                                )
                else:
                    for kh in range(K):
                        rowacc = rpool.tile([128, G, HB, W], cdt)
                        for kw in range(K):
                            kk = kh * K + kw
                            if kw % 2 == 0:
                                in0 = in_pad[:, n, :, kh : kh + HB, kw : kw + W]
                            else:
                                in0 = in_pad1[:, n, :, kh : kh + HB, kw - 1 : kw - 1 + W]
                            in1 = (
                                w_t[:, n, kk].unsqueeze(1).broadcast_to([128, G, HB, W])
                            )
                            if kw == 0:
                                nc.vector.tensor_tensor(
                                    out=rowacc[:], in0=in0, in1=in1, op=mult
                                )
                            else:
                                prod = ppool.tile([128, G, HB, W], cdt)
                                nc.vector.tensor_tensor(
                                    out=prod[:], in0=in0, in1=in1, op=mult
                                )
                                nc.vector.tensor_tensor(
                                    out=rowacc[:], in0=rowacc[:], in1=prod[:], op=add
                                )
                        if kh == 0:
                            # fp16 -> fp32 convert-copy on the idle ACT engine
                            nc.scalar.copy(out=acc[:, n], in_=rowacc[:])
                        else:
                            nc.vector.tensor_tensor(
                                out=acc[:, n], in0=acc[:, n], in1=rowacc[:], op=add
                            )
                dsty = y[n].rearrange(
                    "(g cc) (blk hb) w -> g cc blk (hb w)", g=G, blk=BLK
                )
                for g in range(G):
                    nc.sync.dma_start(
                        out=dsty[g],
                        in_=acc[:, n, g].rearrange("p hb w -> p (hb w)"),
                    )

    nc.compile()
    return nc


def _get_nc(mode=None):
    mode = mode or MODE
    if mode not in _cache:
        _cache[mode] = _build(mode)
    return _cache[mode]


def kernel(input_, weight, _trace=False, _mode=None):
    from concourse.bass_utils import run_bass_kernel_spmd

    nc = _get_nc(_mode)
    input_ = np.ascontiguousarray(input_, dtype=np.float32)
    weight = np.ascontiguousarray(weight, dtype=np.float32)
    in_maps = [
        {
            "input": input_[i * NPC : (i + 1) * NPC],
            "weight": weight[i * NPC : (i + 1) * NPC],
        }
        for i in range(NCORES)
    ]
    res = run_bass_kernel_spmd(nc, in_maps, list(range(NCORES)), trace=_trace)
    _cache["last_result"] = res
    out = np.concatenate([res.results[i]["output"] for i in range(NCORES)], axis=0)
    return out


# revision 7
# speedup vs baseline: 1.6958x; 1.0378x over previous
"""Trainium2 Bass kernel for nn_Aggregation (SAN-style local aggregation).

out[n, g*32+cc, h, w] = sum_{kh,kw} input[n, g*32+cc, h-3+kh, w-3+kw] * weight[n, cc, kh*7+kw, h, w]

Sharding: data-parallel over batch N=16 across 8 NeuronCores (2 images/core).

Per-core layout:
  partition p = cc*4 + blk   (cc in [0,32): weight channel, blk in [0,4): block of 8 output rows)
  in_pad[p][n, g, r, col] = zero-padded input rows [blk*8-3, blk*8+11), cols [-3, 35)
  w_t[p][n, kk, hb, w]    = weight[n, cc, kk, blk*8+hb, w]
  For each tap kk=(kh,kw): acc[p][n,g,hb,w] += in_pad[p][n,g,hb+kh,w+kw] * w_t[p][n,kk,hb,w]
  (weight broadcast over g via stride-0 access pattern)

Mode "fp16row" (default): products and within-row (7-tap) accumulation in fp16
on the DVE at 2x rate; row sums flushed into an fp32 accumulator. A second
input copy shifted by one column keeps odd-kw taps 4B-aligned so the DVE's
2x perf mode stays engaged. Max abs error vs fp32 reference ~7e-4 of absmax.
Mode "fp32": everything fp32 (exact, ~2x slower).
"""

import numpy as np

N, C, H, W = 16, 256, 32, 32
K, PAD = 7, 3
CC, G = 32, 8
KK = K * K
NCORES = 8
NPC = N // NCORES
BLK, HB = 4, 8
R, COLP = HB + 2 * PAD, W + 2 * PAD  # 14, 38

MODE = "fp16row"

_cache = {}


def _build(mode):
    import concourse.bacc as bacc
    import concourse.mybir as mybir
    import concourse.tile as tile

    fp32 = mybir.dt.float32
    fp16 = mybir.dt.float16
    cdt = fp32 if mode == "fp32" else fp16  # compute dtype
    mult = mybir.AluOpType.mult
    add = mybir.AluOpType.add

    nc = bacc.Bacc("TRN2", target_bir_lowering=False, debug=False, num_devices=NCORES)
    x = nc.dram_tensor("input", [NPC, C, H, W], fp32, kind="ExternalInput").ap()
    wgt = nc.dram_tensor("weight", [NPC, CC, KK, H, W], fp32, kind="ExternalInput").ap()
    y = nc.dram_tensor("output", [NPC, C, H, W], fp32, kind="ExternalOutput").ap()

    with tile.TileContext(nc) as tc:
        with (
            tc.tile_pool(name="main", bufs=1) as pool,
            tc.tile_pool(name="prod", bufs=2) as ppool,
            tc.tile_pool(name="rowp", bufs=2) as rpool,
        ):
            in_pad = pool.tile([128, NPC, G, R, COLP], cdt)
            stage = pool.tile([128, NPC, G, R, W], cdt)
            w_t = pool.tile([128, NPC, KK, HB, W], cdt)
            acc = pool.tile([128, NPC, G, HB, W], fp32)
            if mode == "fp16row":
                in_pad1 = pool.tile([128, NPC, G, R, COLP], cdt)

            # Zero only the halo regions: left/right column borders of in_pad,
            # and the top/bottom staging row-bands that flow into in_pad rows.
            # (in_pad1 is fully covered by the shift-copy; its last column is
            # never read.)
            for n in range(NPC):
                nc.gpsimd.memset(in_pad[:, n, :, :, 0:PAD], 0.0)
                nc.gpsimd.memset(in_pad[:, n, :, :, PAD + W : COLP], 0.0)
                # full-partition row bands; the interior DMAs overwrite the
                # rows that are valid for their block, the rest stay zero
                nc.gpsimd.memset(stage[:, n, :, 0:PAD, :], 0.0)
                nc.gpsimd.memset(stage[:, n, :, R - PAD : R, :], 0.0)

            # DMA engine: SWDGE (gpsimd) is required for dtype-cast loads
            dma_in = nc.gpsimd if cdt != fp32 else nc.sync

            for n in range(NPC):
                for blk in range(BLK):
                    # rows of the padded window [blk*8-3, blk*8+11) that exist
                    h0 = max(0, blk * HB - PAD)
                    h1 = min(H, blk * HB + HB + PAD)
                    r0 = h0 - (blk * HB - PAD)
                    dst = stage[blk::BLK, n].rearrange("p g r c -> p g (r c)")[
                        :, :, r0 * W : (r0 + (h1 - h0)) * W
                    ]
                    src = x[n].rearrange("(g cc) h w -> cc g (h w)", g=G)[
                        :, :, h0 * W : h1 * W
                    ]
                    dma_in.dma_start(out=dst, in_=src)
                # place unpadded rows into the column-padded tile (ACT is idle)
                nc.scalar.copy(out=in_pad[:, n, :, :, PAD : PAD + W], in_=stage[:, n])
                if mode == "fp16row":
                    # column-shifted copy: in_pad1[..., c] = in_pad[..., c+1]
                    # keeps odd-kw taps 4B-aligned for the DVE 2x perf mode
                    nc.scalar.copy(
                        out=in_pad1[:, n, :, :, 0 : COLP - 1],
                        in_=in_pad[:, n, :, :, 1:COLP],
                    )
                wsrc = wgt[n].rearrange("cc kk (blk hb) w -> cc blk kk (hb w)", blk=BLK)
                for blk in range(BLK):
                    dma_in.dma_start(
                        out=w_t[blk::BLK, n].rearrange("p kk hb w -> p kk (hb w)"),
                        in_=wsrc[:, blk],
                    )

            for n in range(NPC):
                if mode == "fp32":
                    for kh in range(K):
                        for kw in range(K):
                            kk = kh * K + kw
                            in0 = in_pad[:, n, :, kh : kh + HB, kw : kw + W]
                            in1 = (
                                w_t[:, n, kk].unsqueeze(1).broadcast_to([128, G, HB, W])
                            )
                            if kk == 0:
                                nc.vector.tensor_tensor(
                                    out=acc[:, n], in0=in0, in1=in1, op=mult
                                )
                            else:
                                prod = ppool.tile([128, G, HB, W], cdt)
                                nc.vector.tensor_tensor(
                                    out=prod[:], in0=in0, in1=in1, op=mult
                                )
                                nc.vector.tensor_tensor(
                                    out=acc[:, n], in0=acc[:, n], in1=prod[:], op=add
                                )
                else:
                    for kh in range(K):
                        rowacc = rpool.tile([128, G, HB, W], cdt)
                        for kw in range(K):
                            kk = kh * K + kw
                            if kw % 2 == 0:
                                in0 = in_pad[:, n, :, kh : kh + HB, kw : kw + W]
                            else:
                                in0 = in_pad1[:, n, :, kh : kh + HB, kw - 1 : kw - 1 + W]
                            in1 = (
                                w_t[:, n, kk].unsqueeze(1).broadcast_to([128, G, HB, W])
                            )
                            if kw == 0:
                                nc.vector.tensor_tensor(
                                    out=rowacc[:], in0=in0, in1=in1, op=mult
                                )
                            else:
                                prod = ppool.tile([128, G, HB, W], cdt)
                                nc.vector.tensor_tensor(
                                    out=prod[:], in0=in0, in1=in1, op=mult
                                )
                                nc.vector.tensor_tensor(
                                    out=rowacc[:], in0=rowacc[:], in1=prod[:], op=add
                                )
                        if kh == 0:
                            # fp16 -> fp32 convert-copy on the idle ACT engine
                            nc.scalar.copy(out=acc[:, n], in_=rowacc[:])
                        else:
                            nc.vector.tensor_tensor(
                                out=acc[:, n], in0=acc[:, n], in1=rowacc[:], op=add
                            )
                dsty = y[n].rearrange(
                    "(g cc) (blk hb) w -> g cc blk (hb w)", g=G, blk=BLK
                )
                for g in range(G):
                    nc.sync.dma_start(
                        out=dsty[g],
                        in_=acc[:, n, g].rearrange("p hb w -> p (hb w)"),
                    )

    nc.compile()
    return nc


def _get_nc(mode=None):
    mode = mode or MODE
    if mode not in _cache:
        _cache[mode] = _build(mode)
    return _cache[mode]


def kernel(input_, weight, _trace=False, _mode=None):
    from concourse.bass_utils import run_bass_kernel_spmd

    nc = _get_nc(_mode)
    input_ = np.ascontiguousarray(input_, dtype=np.float32)
    weight = np.ascontiguousarray(weight, dtype=np.float32)
    in_maps = [
        {
            "input": input_[i * NPC : (i + 1) * NPC],
            "weight": weight[i * NPC : (i + 1) * NPC],
        }
        for i in range(NCORES)
    ]
    res = run_bass_kernel_spmd(nc, in_maps, list(range(NCORES)), trace=_trace)
    _cache["last_result"] = res
    out = np.concatenate([res.results[i]["output"] for i in range(NCORES)], axis=0)
    return out
